# revision 20
# baseline (speedup 1.0000x reference)
"""DeformConv3D on 8 TRN2 cores — fused SINGLE-LAUNCH kernel, L-sharded.

Core k owns view z-slices l in {2k, 2k+1} (all 128 (b,c) planes, full HxW):
  P1: offset conv for exactly the 12 "units" (ch, l2) = divmod(6k+j, 16),
      j in [0,6) x 2 batches, whose raw conv outputs interleave into the
      coords of view-l 2k/2k+1 (torch .view scramble is l-local!).
      K packs (c, z-tap-pair) = 128; writes bf16 slab scratch in DRAM.
  P2: de-interleave slab (stride-3) -> per-dim displacement f = clamp(off+g)-g
      -> separable 5-tap tent gather on DVE (bf16), ACT computes tents.
      DVE work is emitted as interleaved multi-stream waves (3 z-tap
      streams, then the 4th z-tap as 2 sy-streams) with ping-pong
      accumulator slots: measured DVE op cost drops ~2x when consecutive
      ops avoid same-buffer RMW and reuse buffers only at distance >= 4.
  P3: partial main conv: core k's deformed z-slices contribute to out
      l in [2k-1, 2k+2]; partials are output as-is (f32) and the HOST
      sums overlapping partials across cores and adds the bias.
No collectives, no cross-core traffic; one launch total. DMA triggers
cost ~5us each on HW regardless of size, so stores are batched: P1
accumulates each unit's full plane in SBUF (1 scratch DMA per unit),
P2 writes defd as one [128, HW] tensor per z-slice, P3 loads each
halo band with a single DMA and batches 4 PSUM chunks per output DMA.
"""
import sys
from contextlib import ExitStack

import numpy as np

sys.path.insert(0, "/opt/trn_rl_repo")
from concourse import bass, bacc, tile, mybir
from concourse.bass_utils import run_bass_kernel_spmd

F32 = mybir.dt.float32
BF16 = mybir.dt.bfloat16
ALU = mybir.AluOpType
AF = mybir.ActivationFunctionType

B, C, L, H, W = 2, 64, 16, 96, 96
HW = H * W
NCORES = 8
TAPS = (-2, -1, 0, 1, 2)
ROWCH = [(r, 5) for r in range(0, 95, 5)] + [(95, 1)]   # P1/P3 N-chunks
HC = 16                                                  # P2 rows per chunk
NHC = H // HC

_nc_cache = None


def build_program():
    nc = bacc.Bacc("TRN2", target_bir_lowering=False, debug=False,
                   num_devices=NCORES)
    x1 = nc.dram_tensor("x1", [B, 6, 3, C, 98, 98], BF16,
                        kind="ExternalInput").ap()
    x2 = nc.dram_tensor("x2", [128, 6, 100, 100], BF16,
                        kind="ExternalInput").ap()
    w1p = nc.dram_tensor("w1p", [6, 9, 128, 64], BF16,
                         kind="ExternalInput").ap()
    w1s = nc.dram_tensor("w1s", [6, 9, 64, 64], BF16,
                         kind="ExternalInput").ap()
    w3a = nc.dram_tensor("w3a", [9, 128, 64], BF16, kind="ExternalInput").ap()
    w3b = nc.dram_tensor("w3b", [9, 128, 64], BF16, kind="ExternalInput").ap()
    w3s2 = nc.dram_tensor("w3s2", [9, 64, 64], BF16, kind="ExternalInput").ap()
    w3s0 = nc.dram_tensor("w3s0", [9, 64, 64], BF16, kind="ExternalInput").ap()
    gy = nc.dram_tensor("gy", [128, HC * W], BF16, kind="ExternalInput").ap()
    gx = nc.dram_tensor("gx", [128, HC * W], BF16, kind="ExternalInput").ap()
    lb = nc.dram_tensor("lb", [128, 2], F32, kind="ExternalInput").ap()
    lbz = nc.dram_tensor("lbz", [128, 10], F32, kind="ExternalInput").ap()
    pout = nc.dram_tensor("pout", [8, 64, HW], F32, kind="ExternalOutput").ap()

    with tile.TileContext(nc) as tc:
        with tc.tile_pool(name="dram", bufs=1, space="DRAM") as dram:
            # split per l-half so P2 lv=0 only depends on P1 units 0-2
            scratchA = dram.tile([128, 3 * HW], BF16)
            scratchB = dram.tile([128, 3 * HW], BF16)
            defds = [dram.tile([128, HW], BF16, name=f"defd{lv}")
                     for lv in range(2)]

            # ---------------- P1 + P2 share one scope so the tile scheduler
            # can overlap P2 (DVE) with P1's trailing units (PE) ------------
            ctx12 = ExitStack()
            p2c = ctx12.enter_context(tc.tile_pool(name="p2c", bufs=1))
            p2win = ctx12.enter_context(tc.tile_pool(name="p2win", bufs=1))
            p2sl = ctx12.enter_context(tc.tile_pool(name="p2sl", bufs=1))
            p2k = ctx12.enter_context(tc.tile_pool(name="p2k", bufs=1))
            ctx1 = ExitStack()
            p1w = ctx1.enter_context(tc.tile_pool(name="p1w", bufs=1))
            p1x = ctx1.enter_context(tc.tile_pool(name="p1x", bufs=1))
            p1ps = ctx1.enter_context(
                tc.tile_pool(name="p1ps", bufs=2, space="PSUM"))
            p1o = ctx1.enter_context(tc.tile_pool(name="p1o", bufs=1))
            if True:
                w1pt = p1w.tile([128, 6, 9, 64], BF16)
                nc.sync.dma_start(w1pt[:], w1p.rearrange("j g p m -> p j g m"))
                w1st = p1w.tile([64, 6, 9, 64], BF16)
                nc.sync.dma_start(w1st[:], w1s.rearrange("j g p m -> p j g m"))
                for j in range(6):
                    for b in range(B):
                        # xu[0:64,0]=z0, xu[64:128,0]=z1, xu[0:64,1]=z2
                        xu = p1x.tile([128, 2, 98, 98], BF16, tag="xu")
                        nc.sync.dma_start(
                            xu[0:64], x1[b, j, 0:3:2].rearrange("z c y x -> c z y x"))
                        nc.sync.dma_start(
                            xu[64:128, 0:1],
                            x1[b, j, 1:2].rearrange("z c y x -> c z y x"))
                        ob = p1o.tile([64, HW], BF16, tag="ob1")
                        for (r0, rn) in ROWCH:
                            n = rn * W
                            ps = p1ps.tile([64, 480], F32, tag="ps1")
                            g = 0
                            for dy in range(3):
                                for dx in range(3):
                                    nc.tensor.matmul(
                                        ps[:, :n], w1pt[:, j, g, :],
                                        xu[:, 0, dy + r0:dy + r0 + rn, dx:dx + W],
                                        start=(g == 0), stop=False)
                                    g += 1
                            g = 0
                            for dy in range(3):
                                for dx in range(3):
                                    nc.tensor.matmul(
                                        ps[:, :n], w1st[:, j, g, :],
                                        xu[0:64, 1, dy + r0:dy + r0 + rn, dx:dx + W],
                                        start=False, stop=(g == 8))
                                    g += 1
                            nc.scalar.copy(ob[:, r0 * W:r0 * W + n], ps[:, :n])
                        scr = scratchA if j < 3 else scratchB
                        jl = j % 3
                        nc.sync.dma_start(
                            scr[b * 64:(b + 1) * 64, jl * HW:(jl + 1) * HW],
                            ob[:])

            # ---------------- P2: tent gather -> defd ----------------
            if True:
                gyt = p2c.tile([128, HC * W], BF16)
                nc.sync.dma_start(gyt[:], gy)
                gxt = p2c.tile([128, HC * W], BF16)
                nc.sync.dma_start(gxt[:], gx)
                lbt = p2c.tile([128, 2], F32)
                nc.sync.dma_start(lbt[:], lb)
                lbzt = p2c.tile([128, 10], F32)
                nc.sync.dma_start(lbzt[:], lbz)
                cb = p2c.tile([128, 3], F32)
                nc.vector.memset(cb[:, 0:1], -2.0)
                nc.vector.memset(cb[:, 1:2], -1.0)
                nc.vector.memset(cb[:, 2:3], 2.0)
                # bias AP for value -t (t in TAPS): +2,+1,0,-1,-2
                bias_of = {-2: cb[:, 2:3], -1: 1.0, 0: 0.0,
                           1: cb[:, 1:2], 2: cb[:, 0:1]}
                neg1 = cb[:, 1:2]

                NP = HC * W   # 1536
                for lv in range(2):
                    scr = scratchA if lv == 0 else scratchB
                    for hc in range(NHC):
                        h0 = HC * hc
                        win = p2win.tile([128, 5, 20, 100], BF16, tag="win")
                        nc.sync.dma_start(win[:],
                                          x2[:, lv:lv + 5, h0:h0 + 20, :])
                        # slab tail (cols 3NP..) hosts the 3rd wave-1 dz
                        slab = p2sl.tile([128, 3 * NP + 2048], BF16,
                                         tag="slab")
                        nc.sync.dma_start(
                            slab[:, 0:3 * NP],
                            scr[:, hc * 3 * NP:(hc + 1) * 3 * NP])
                        sv = slab[:, 0:3 * NP].rearrange("p (n k) -> p k n",
                                                         k=3)
                        offz = p2k.tile([128, NP], F32, tag="offz")
                        offy = p2k.tile([128, NP], F32, tag="offy")
                        offx = p2k.tile([128, NP], F32, tag="offx")
                        # de-interleave on ACT (keeps DVE free)
                        nc.scalar.activation(offz[:], sv[:, 0], AF.Copy)
                        nc.scalar.activation(offy[:], sv[:, 1], AF.Copy)
                        nc.scalar.activation(offx[:], sv[:, 2], AF.Copy)
                        # s_z = clamp(off_z + l, 0, 15) = min(Relu(off_z + l), 15)
                        nc.scalar.activation(offz[:], offz[:], AF.Relu,
                                             bias=lbt[:, lv:lv + 1])
                        nc.gpsimd.tensor_scalar(offz[:], offz[:], 15.0, None,
                                                ALU.min)
                        # f_y = clamp(off_y + gy + h0, 0, 95) - h0 - gy
                        nc.gpsimd.tensor_tensor(offy[:], offy[:], gyt[:], ALU.add)
                        nc.gpsimd.tensor_scalar(offy[:], offy[:], float(h0), 0.0,
                                                ALU.add, ALU.max)
                        nc.gpsimd.tensor_scalar(offy[:], offy[:], 95.0, float(h0),
                                                ALU.min, ALU.subtract)
                        nc.gpsimd.tensor_tensor(offy[:], offy[:], gyt[:],
                                                ALU.subtract)
                        # f_x = clamp(off_x + gx, 0, 95) - gx
                        nc.gpsimd.tensor_tensor(offx[:], offx[:], gxt[:], ALU.add)
                        nc.gpsimd.tensor_scalar(offx[:], offx[:], 0.0, 95.0,
                                                ALU.max, ALU.min)
                        nc.gpsimd.tensor_tensor(offx[:], offx[:], gxt[:],
                                                ALU.subtract)
                        # tents
                        u = p2k.tile([128, NP], BF16, tag="u")
                        # PWL knots: c_k = clamp(f_x - k, 0, 1), k in -2..1
                        cks, lamy = [], []
                        for ik, kk in enumerate((-2, -1, 0, 1)):
                            ck = p2k.tile([128, NP], BF16, tag=f"ck{ik}")
                            nc.scalar.activation(ck[:], offx[:], AF.Relu,
                                                 bias=bias_of[kk])
                            nc.gpsimd.tensor_scalar(ck[:], ck[:], 1.0, None,
                                                    ALU.min)
                            cks.append(ck)
                        for t in TAPS:
                            nc.scalar.activation(u[:], offy[:], AF.Abs,
                                                 bias=bias_of[t])
                            lt = p2k.tile([128, NP], BF16, tag=f"lamy{t}")
                            nc.scalar.activation(lt[:], u[:], AF.Relu,
                                                 bias=1.0, scale=neg1)
                            lamy.append(lt)
                        # interleaved-wave slot tiles (see wave emission
                        # below). offy/offx die after cks/lamy -> their
                        # f32 storage hosts 4 bf16 slots via bitcast.
                        lamz = p2k.tile([128, NP], BF16, tag="lamz")
                        accb = p2k.tile([128, NP], BF16, tag="accb")
                        offyB = offy[:].bitcast(BF16)
                        offxB = offx[:].bitcast(BF16)
                        sl0 = p2k.tile([128, NP], BF16, tag="tmpi")
                        sl1 = p2k.tile([128, NP], BF16, tag="prod")
                        sl2 = p2k.tile([128, NP], BF16, tag="tmpb")
                        sl3 = p2k.tile([128, NP], BF16, tag="tmpz")
                        sl4 = p2k.tile([128, NP], BF16, tag="nsl0")
                        sl5 = p2k.tile([128, NP], BF16, tag="nsl1")
                        sl6 = p2k.tile([128, NP], BF16, tag="nsl2")
                        sl7 = p2k.tile([128, NP], BF16, tag="nsl3")
                        sl8 = p2k.tile([128, NP], BF16, tag="nsl4")
                        sl9 = p2k.tile([128, NP], BF16, tag="nsl5")
                        t_pi = [[sl0[:], sl1[:]], [sl2[:], sl3[:]],
                                [offyB[:, 0:NP], offyB[:, NP:2 * NP]]]
                        t_tb = [[offxB[:, 0:NP], offxB[:, NP:2 * NP]],
                                [sl4[:], sl5[:]], [sl6[:], sl7[:]]]
                        t_pr = [sl8[:], sl9[:]]
                        # slots for wave-2's second z-stream (z-tap +2):
                        # x-PWL moved off GPSIMD (strided reads are slow
                        # on the software DSP) onto DVE as stream B
                        tmpi_g = p2k.tile([128, NP], BF16, tag="tmpi_g")
                        prod_g = p2k.tile([128, NP], BF16, tag="prod_g")
                        tmpb_g = p2k.tile([128, NP], BF16, tag="tmpb_g")
                        dzg = p2k.tile([128, 20, 99], BF16, tag="dzg")
                        # ---- wave 1: z-taps 0..2, 3 DVE streams ----
                        dzv = [slab[:, o:o + 1980].rearrange(
                                   "p (y x) -> p y x", y=20)
                               for o in (0, 1980, 3 * NP)]
                        for s in range(3):
                            nc.vector.tensor_tensor(
                                dzv[s], win[:, s, :, 1:100],
                                win[:, s, :, 0:99], ALU.subtract)
                        for iy, sy in enumerate(TAPS):
                            for ik in range(4):
                                def msl(s):
                                    return dzv[s][:, sy + 2:sy + 2 + HC,
                                                  ik:ik + W]

                                def asr(s):
                                    if ik == 0:
                                        return win[:, s, sy + 2:sy + 2 + HC,
                                                   0:W]
                                    return t_pi[s][(ik + 1) % 2]

                                nc.vector.tensor_tensor(
                                    t_pr[0], cks[ik][:], msl(0), ALU.mult)
                                nc.vector.tensor_tensor(
                                    t_pr[1], cks[ik][:], msl(1), ALU.mult)
                                nc.vector.tensor_tensor(
                                    t_pi[0][ik % 2], asr(0), t_pr[0],
                                    ALU.add)
                                nc.vector.tensor_tensor(
                                    t_pr[0], cks[ik][:], msl(2), ALU.mult)
                                nc.vector.tensor_tensor(
                                    t_pi[1][ik % 2], asr(1), t_pr[1],
                                    ALU.add)
                                nc.vector.tensor_tensor(
                                    t_pi[2][ik % 2], asr(2), t_pr[0],
                                    ALU.add)
                            # y-combine on GPSIMD (contiguous ops only —
                            # its strided access is slow, contiguous is
                            # fine) to keep DVE on the x-PWL stream.
                            # gps-private prod slots to avoid cross-engine
                            # races on t_pr.
                            fin = [t_pi[s][1] for s in range(3)]
                            if iy == 0:
                                for s in range(3):
                                    nc.gpsimd.tensor_tensor(
                                        t_tb[s][0], lamy[0][:], fin[s],
                                        ALU.mult)
                            else:
                                for s in range(3):
                                    gpr = tmpi_g if s % 2 == 0 else prod_g
                                    nc.gpsimd.tensor_tensor(
                                        gpr[:], lamy[iy][:], fin[s],
                                        ALU.mult)
                                    nc.gpsimd.tensor_tensor(
                                        t_tb[s][iy % 2],
                                        t_tb[s][(iy + 1) % 2],
                                        gpr[:], ALU.add)
                        # wave-1 z-products: lamz_z staggered on ACT
                        # (final tmpb of stream s lives in t_tb[s][0])
                        psl = [t_pi[0][0], t_pi[0][1], t_tb[0][1]]
                        for s in range(3):
                            nc.scalar.activation(
                                u[:], offz[:], AF.Abs,
                                bias=lbzt[:, lv * 5 + s:lv * 5 + s + 1])
                            nc.scalar.activation(lamz[:], u[:], AF.Relu,
                                                 bias=1.0, scale=neg1)
                            nc.vector.tensor_tensor(
                                psl[s], lamz[:], t_tb[s][0], ALU.mult)
                        s01 = t_tb[1][0]
                        s012 = t_tb[1][1]
                        nc.vector.tensor_tensor(s01, psl[0], psl[1],
                                                ALU.add)
                        nc.vector.tensor_tensor(s012, s01, psl[2], ALU.add)
                        # ---- wave 2: z-taps 3,4 as 2 interleaved z-streams
                        # stream A slots: t_pi[1] pair, tb pp t_pi[0];
                        # stream B slots: tmpi_g/prod_g pair, tb pp
                        # tmpb_g + dzg (flat view)
                        nc.vector.tensor_tensor(
                            dzv[0], win[:, 3, :, 1:100], win[:, 3, :, 0:99],
                            ALU.subtract)
                        nc.vector.tensor_tensor(
                            dzv[1], win[:, 4, :, 1:100], win[:, 4, :, 0:99],
                            ALU.subtract)
                        tgb = dzg[:].rearrange("p y x -> p (y x)")[:, 0:NP]
                        w2pi = [t_pi[1], [tmpi_g[:], prod_g[:]]]
                        w2tb = [t_pi[0], [tmpb_g[:], tgb]]
                        for iy, sy in enumerate(TAPS):
                            for ik in range(4):
                                for zz in range(2):
                                    nc.vector.tensor_tensor(
                                        t_pr[zz], cks[ik][:],
                                        dzv[zz][:, sy + 2:sy + 2 + HC,
                                                ik:ik + W], ALU.mult)
                                for zz in range(2):
                                    src = (win[:, 3 + zz,
                                               sy + 2:sy + 2 + HC, 0:W]
                                           if ik == 0
                                           else w2pi[zz][(ik + 1) % 2])
                                    nc.vector.tensor_tensor(
                                        w2pi[zz][ik % 2], src, t_pr[zz],
                                        ALU.add)
                            for zz in range(2):
                                if iy == 0:
                                    nc.vector.tensor_tensor(
                                        w2tb[zz][0], lamy[0][:],
                                        w2pi[zz][1], ALU.mult)
                                else:
                                    nc.vector.tensor_tensor(
                                        t_pr[zz], lamy[iy][:],
                                        w2pi[zz][1], ALU.mult)
                                    nc.vector.tensor_tensor(
                                        w2tb[zz][iy % 2],
                                        w2tb[zz][(iy + 1) % 2],
                                        t_pr[zz], ALU.add)
                        # final combine: + lamz3*tbA + lamz4*tbB
                        # (finals in w2tb[zz][0] since last iy=4 is even)
                        nc.scalar.activation(
                            u[:], offz[:], AF.Abs,
                            bias=lbzt[:, lv * 5 + 3:lv * 5 + 4])
                        nc.scalar.activation(lamz[:], u[:], AF.Relu,
                                             bias=1.0, scale=neg1)
                        nc.vector.tensor_tensor(
                            t_pi[1][0], lamz[:], w2tb[0][0], ALU.mult)
                        nc.vector.tensor_tensor(
                            t_pi[1][1], s012, t_pi[1][0], ALU.add)
                        nc.scalar.activation(
                            u[:], offz[:], AF.Abs,
                            bias=lbzt[:, lv * 5 + 4:lv * 5 + 5])
                        nc.scalar.activation(lamz[:], u[:], AF.Relu,
                                             bias=1.0, scale=neg1)
                        nc.vector.tensor_tensor(
                            t_pi[2][0], lamz[:], w2tb[1][0], ALU.mult)
                        nc.vector.tensor_tensor(
                            accb[:], t_pi[1][1], t_pi[2][0], ALU.add)
                        nc.sync.dma_start(
                            defds[lv][:, hc * HC * W:(hc + 1) * HC * W],
                            accb[:])

            # close P1 pools only now (after all P2 tags are allocated)
            # so P3 tiles reuse P1's SBUF, not P2's
            ctx1.close()
            # ---------- P3: banded partial main conv, overlapped with P2 ----
            p3w = ctx12.enter_context(tc.tile_pool(name="p3w", bufs=1))
            p3d = ctx12.enter_context(tc.tile_pool(name="p3d", bufs=2))
            p3ps = ctx12.enter_context(
                tc.tile_pool(name="p3ps", bufs=2, space="PSUM"))
            p3o = ctx12.enter_context(tc.tile_pool(name="p3o", bufs=2))
            if True:
                w3at = p3w.tile([128, 9, 64], BF16)
                nc.sync.dma_start(w3at[:], w3a.rearrange("g p m -> p g m"))
                w3bt = p3w.tile([128, 9, 64], BF16)
                nc.sync.dma_start(w3bt[:], w3b.rearrange("g p m -> p g m"))
                w3s2t = p3w.tile([64, 9, 64], BF16)
                nc.sync.dma_start(w3s2t[:], w3s2.rearrange("g p m -> p g m"))
                w3s0t = p3w.tile([64, 9, 64], BF16)
                nc.sync.dma_start(w3s0t[:], w3s0.rearrange("g p m -> p g m"))
                for hc3 in range(NHC):
                    y0 = HC * hc3
                    ys = max(0, y0 - 1)
                    ye = min(H, y0 + HC + 1)
                    for b in range(B):
                        dcA = p3d.tile([128, 18, 98], BF16, tag=f"dcA{b}")
                        nc.scalar.memzero(dcA[:].rearrange("p y x -> p (y x)"))
                        dcB = p3d.tile([64, 18, 98], BF16, tag=f"dcB{b}")
                        nc.scalar.memzero(dcB[:].rearrange("p y x -> p (y x)"))
                        for lv in range(2):
                            for dst in ([dcA[64 * lv:64 * lv + 64]]
                                        + ([dcB[0:64]] if lv == 1 else [])):
                                nc.sync.dma_start(
                                    dst[:, ys - y0 + 1:ye - y0 + 1, 1:97],
                                    defds[lv][b * 64:(b + 1) * 64,
                                              ys * W:ye * W]
                                    .rearrange("c (y x) -> c y x", y=ye - ys))
                        specs = [(w3s2t, dcA[0:64], 64),
                                 (w3at, dcA[:], 128),
                                 (w3bt, dcA[:], 128),
                                 (w3s0t, dcB[0:64], 64)]
                        for li, (wt, dct, kk) in enumerate(specs):
                            ob = p3o.tile([64, HC * W], F32,
                                          tag=f"ob3{li % 2}")
                            for qi, q0 in enumerate((0, 4, 8, 12)):
                                n = 4 * W
                                ps = p3ps.tile([64, 384], F32,
                                               tag=f"ps3{li % 2}")
                                g = 0
                                for dy in range(3):
                                    for dx in range(3):
                                        nc.tensor.matmul(
                                            ps[:, :n], wt[:, g, :],
                                            dct[:, dy + q0:dy + q0 + 4,
                                                dx:dx + W],
                                            start=(g == 0), stop=(g == 8))
                                        g += 1
                                nc.scalar.copy(
                                    ob[:, qi * n:(qi + 1) * n], ps[:, :n])
                            nc.sync.dma_start(
                                pout[li * 2 + b, :,
                                     y0 * W:y0 * W + HC * W], ob[:])
            ctx12.close()
    nc.finalize()
    return nc


def kernel(x, w_off, w_conv, b_conv):
    global _nc_cache
    import ml_dtypes
    x = np.asarray(x, dtype=np.float32)
    w_off = np.asarray(w_off, dtype=np.float32)
    w_conv = np.asarray(w_conv, dtype=np.float32)
    b_conv = np.asarray(b_conv, dtype=np.float32)

    if _nc_cache is None:
        _nc_cache = build_program()

    bf = ml_dtypes.bfloat16
    # P1 source: pad z/y/x by 1
    xp1 = np.zeros((B, C, L + 2, 98, 98), bf)
    xp1[:, :, 1:L + 1, 1:H + 1, 1:W + 1] = x.astype(bf)
    # P2 source: pad z/y/x by 2
    xp2 = np.zeros((B, C, L + 4, 100, 100), bf)
    xp2[:, :, 2:L + 2, 2:H + 2, 2:W + 2] = x.astype(bf)

    woff_r = w_off.reshape(64, 3, C, 3, 3, 3)      # [m', ch, c, dz, dy, dx]
    wt_off = np.ascontiguousarray(
        np.transpose(woff_r, (1, 4, 5, 3, 2, 0)))  # [ch, dy, dx, dz, c, m']
    wc_t = np.transpose(w_conv, (3, 4, 2, 1, 0))   # [dy, dx, dz, c, m]
    w3a = np.ascontiguousarray(
        wc_t[:, :, 1:3].reshape(9, 128, 64)).astype(bf)
    w3b = np.ascontiguousarray(
        wc_t[:, :, 0:2].reshape(9, 128, 64)).astype(bf)
    w3s2 = np.ascontiguousarray(wc_t[:, :, 2].reshape(9, 64, 64)).astype(bf)
    w3s0 = np.ascontiguousarray(wc_t[:, :, 0].reshape(9, 64, 64)).astype(bf)

    gyt = np.broadcast_to(
        np.repeat(np.arange(HC, dtype=np.float32), W)[None], (128, HC * W))
    gxt = np.broadcast_to(
        np.tile(np.arange(W, dtype=np.float32), HC)[None], (128, HC * W))
    gyt = np.ascontiguousarray(gyt).astype(bf)
    gxt = np.ascontiguousarray(gxt).astype(bf)

    in_maps = []
    for k in range(NCORES):
        units = [divmod(6 * k + j, 16) for j in range(6)]   # (ch, l2)
        x1 = np.empty((B, 6, 3, C, 98, 98), bf)
        for j, (ch, l2) in enumerate(units):
            x1[:, j] = np.transpose(xp1[:, :, l2:l2 + 3], (0, 2, 1, 3, 4))
        x2 = np.ascontiguousarray(
            xp2[:, :, 2 * k:2 * k + 6].reshape(128, 6, 100, 100))
        w1p = np.empty((6, 9, 128, 64), bf)
        w1s = np.empty((6, 9, 64, 64), bf)
        for j, (ch, l2) in enumerate(units):
            w1p[j] = wt_off[ch, :, :, 0:2].reshape(9, 128, 64)
            w1s[j] = wt_off[ch, :, :, 2].reshape(9, 64, 64)
        lbv = np.array([2 * k, 2 * k + 1], np.float32)
        lb = np.broadcast_to(lbv[None], (128, 2)).copy()
        # lbz[:, lv*5+iz] = -(2k + lv + sz), sz = TAPS[iz]
        lbzv = np.array([-(2 * k + lv + sz) for lv in range(2) for sz in TAPS],
                        np.float32)
        lbz = np.broadcast_to(lbzv[None], (128, 10)).copy()
        in_maps.append({
            "x1": x1, "x2": x2, "w1p": w1p, "w1s": w1s,
            "w3a": w3a, "w3b": w3b, "w3s2": w3s2, "w3s0": w3s0,
            "gy": gyt, "gx": gxt, "lb": lb, "lbz": lbz,
        })

    res = run_bass_kernel_spmd(_nc_cache, in_maps, list(range(NCORES)))

    out = np.zeros((B, 64, L, H, W), np.float32)
    for k in range(NCORES):
        po = res.results[k]["pout"]        # [8, 64, HW]
        for li in range(4):
            lg = 2 * k - 1 + li
            if 0 <= lg < L:
                for b in range(B):
                    out[b, :, lg] += po[li * 2 + b].reshape(64, H, W)
    out += b_conv[None, :, None, None, None]
    return out



# revision 21
# speedup vs baseline: 1.1352x; 1.1352x over previous
"""DeformConv3D on 8 TRN2 cores — fused SINGLE-LAUNCH kernel, L-sharded.

Core k owns view z-slices l in {2k, 2k+1} (all 128 (b,c) planes, full HxW):
  P1: offset conv for exactly the 12 "units" (ch, l2) = divmod(6k+j, 16),
      j in [0,6) x 2 batches, whose raw conv outputs interleave into the
      coords of view-l 2k/2k+1 (torch .view scramble is l-local!).
      K packs (c, z-tap-pair) = 128; writes bf16 slab scratch in DRAM.
  P2: de-interleave slab (stride-3) -> per-dim displacement f = clamp(off+g)-g
      -> separable 5-tap tent gather on DVE (bf16), ACT computes tents.
      DVE work is emitted as interleaved multi-stream waves (3 z-tap
      streams, then the 4th z-tap as 2 sy-streams) with ping-pong
      accumulator slots: measured DVE op cost drops ~2x when consecutive
      ops avoid same-buffer RMW and reuse buffers only at distance >= 4.
  P3: partial main conv: core k's deformed z-slices contribute to out
      l in [2k-1, 2k+2]; partials are output as-is (f32) and the HOST
      sums overlapping partials across cores and adds the bias.
No collectives, no cross-core traffic; one launch total. DMA triggers
cost ~5us each on HW regardless of size, so stores are batched: P1
accumulates each unit's full plane in SBUF (1 scratch DMA per unit),
P2 writes defd as one [128, HW] tensor per z-slice, P3 loads each
halo band with a single DMA and batches 4 PSUM chunks per output DMA.
"""
import sys
from contextlib import ExitStack

import numpy as np

sys.path.insert(0, "/opt/trn_rl_repo")
from concourse import bass, bacc, tile, mybir
from concourse.bass_utils import run_bass_kernel_spmd

F32 = mybir.dt.float32
BF16 = mybir.dt.bfloat16
ALU = mybir.AluOpType
AF = mybir.ActivationFunctionType

B, C, L, H, W = 2, 64, 16, 96, 96
HW = H * W
NCORES = 8
TAPS = (-2, -1, 0, 1, 2)
ROWCH = [(r, 5) for r in range(0, 95, 5)] + [(95, 1)]   # P1/P3 N-chunks
HC = 16                                                  # P2 rows per chunk
NHC = H // HC

_nc_cache = None


def build_program():
    nc = bacc.Bacc("TRN2", target_bir_lowering=False, debug=False,
                   num_devices=NCORES)
    x1 = nc.dram_tensor("x1", [B, 6, 3, C, 98, 98], BF16,
                        kind="ExternalInput").ap()
    x2 = nc.dram_tensor("x2", [128, 6, 100, 100], BF16,
                        kind="ExternalInput").ap()
    w1p = nc.dram_tensor("w1p", [6, 9, 128, 64], BF16,
                         kind="ExternalInput").ap()
    w1s = nc.dram_tensor("w1s", [6, 9, 64, 64], BF16,
                         kind="ExternalInput").ap()
    w3a = nc.dram_tensor("w3a", [9, 128, 64], BF16, kind="ExternalInput").ap()
    w3b = nc.dram_tensor("w3b", [9, 128, 64], BF16, kind="ExternalInput").ap()
    w3s2 = nc.dram_tensor("w3s2", [9, 64, 64], BF16, kind="ExternalInput").ap()
    w3s0 = nc.dram_tensor("w3s0", [9, 64, 64], BF16, kind="ExternalInput").ap()
    gy = nc.dram_tensor("gy", [128, HC * W], BF16, kind="ExternalInput").ap()
    gx = nc.dram_tensor("gx", [128, HC * W], BF16, kind="ExternalInput").ap()
    lb = nc.dram_tensor("lb", [128, 2], F32, kind="ExternalInput").ap()
    lbz = nc.dram_tensor("lbz", [128, 10], F32, kind="ExternalInput").ap()
    pout = nc.dram_tensor("pout", [8, 64, HW], F32, kind="ExternalOutput").ap()

    with tile.TileContext(nc) as tc:
        with tc.tile_pool(name="dram", bufs=1, space="DRAM") as dram:
            # split per l-half so P2 lv=0 only depends on P1 units 0-2
            scratchA = dram.tile([128, 3 * HW], BF16)
            scratchB = dram.tile([128, 3 * HW], BF16)
            defds = [dram.tile([128, HW], BF16, name=f"defd{lv}")
                     for lv in range(2)]

            # ---------------- P1 + P2 share one scope so the tile scheduler
            # can overlap P2 (DVE) with P1's trailing units (PE) ------------
            ctx12 = ExitStack()
            p2c = ctx12.enter_context(tc.tile_pool(name="p2c", bufs=1))
            p2win = ctx12.enter_context(tc.tile_pool(name="p2win", bufs=1))
            p2sl = ctx12.enter_context(tc.tile_pool(name="p2sl", bufs=1))
            p2k = ctx12.enter_context(tc.tile_pool(name="p2k", bufs=1))
            ctx1 = ExitStack()
            p1w = ctx1.enter_context(tc.tile_pool(name="p1w", bufs=1))
            p1x = ctx1.enter_context(tc.tile_pool(name="p1x", bufs=1))
            p1ps = ctx1.enter_context(
                tc.tile_pool(name="p1ps", bufs=2, space="PSUM"))
            p1o = ctx1.enter_context(tc.tile_pool(name="p1o", bufs=1))
            if True:
                w1pt = p1w.tile([128, 6, 9, 64], BF16)
                nc.sync.dma_start(w1pt[:], w1p.rearrange("j g p m -> p j g m"))
                w1st = p1w.tile([64, 6, 9, 64], BF16)
                nc.sync.dma_start(w1st[:], w1s.rearrange("j g p m -> p j g m"))
                for j in range(6):
                    for b in range(B):
                        # xu[0:64,0]=z0, xu[64:128,0]=z1, xu[0:64,1]=z2
                        xu = p1x.tile([128, 2, 98, 98], BF16, tag="xu")
                        nc.sync.dma_start(
                            xu[0:64], x1[b, j, 0:3:2].rearrange("z c y x -> c z y x"))
                        nc.sync.dma_start(
                            xu[64:128, 0:1],
                            x1[b, j, 1:2].rearrange("z c y x -> c z y x"))
                        ob = p1o.tile([64, HW], BF16, tag="ob1")
                        for (r0, rn) in ROWCH:
                            n = rn * W
                            ps = p1ps.tile([64, 480], F32, tag="ps1")
                            g = 0
                            for dy in range(3):
                                for dx in range(3):
                                    nc.tensor.matmul(
                                        ps[:, :n], w1pt[:, j, g, :],
                                        xu[:, 0, dy + r0:dy + r0 + rn, dx:dx + W],
                                        start=(g == 0), stop=False)
                                    g += 1
                            g = 0
                            for dy in range(3):
                                for dx in range(3):
                                    nc.tensor.matmul(
                                        ps[:, :n], w1st[:, j, g, :],
                                        xu[0:64, 1, dy + r0:dy + r0 + rn, dx:dx + W],
                                        start=False, stop=(g == 8))
                                    g += 1
                            nc.scalar.copy(ob[:, r0 * W:r0 * W + n], ps[:, :n])
                        scr = scratchA if j < 3 else scratchB
                        jl = j % 3
                        nc.sync.dma_start(
                            scr[b * 64:(b + 1) * 64, jl * HW:(jl + 1) * HW],
                            ob[:])

            # ---------------- P2: tent gather -> defd ----------------
            if True:
                gyt = p2c.tile([128, HC * W], BF16)
                nc.sync.dma_start(gyt[:], gy)
                gxt = p2c.tile([128, HC * W], BF16)
                nc.sync.dma_start(gxt[:], gx)
                lbt = p2c.tile([128, 2], F32)
                nc.sync.dma_start(lbt[:], lb)
                lbzt = p2c.tile([128, 10], F32)
                nc.sync.dma_start(lbzt[:], lbz)
                cb = p2c.tile([128, 3], F32)
                nc.vector.memset(cb[:, 0:1], -2.0)
                nc.vector.memset(cb[:, 1:2], -1.0)
                nc.vector.memset(cb[:, 2:3], 2.0)
                # bias AP for value -t (t in TAPS): +2,+1,0,-1,-2
                bias_of = {-2: cb[:, 2:3], -1: 1.0, 0: 0.0,
                           1: cb[:, 1:2], 2: cb[:, 0:1]}
                neg1 = cb[:, 1:2]

                NP = HC * W   # 1536
                for lv in range(2):
                    scr = scratchA if lv == 0 else scratchB
                    for hc in range(NHC):
                        h0 = HC * hc
                        win = p2win.tile([128, 5, 20, 100], BF16, tag="win")
                        nc.sync.dma_start(win[:],
                                          x2[:, lv:lv + 5, h0:h0 + 20, :])
                        # slab tail (cols 3NP..) hosts the 3rd wave-1 dz
                        slab = p2sl.tile([128, 3 * NP + 2048], BF16,
                                         tag="slab")
                        nc.sync.dma_start(
                            slab[:, 0:3 * NP],
                            scr[:, hc * 3 * NP:(hc + 1) * 3 * NP])
                        sv = slab[:, 0:3 * NP].rearrange("p (n k) -> p k n",
                                                         k=3)
                        offz = p2k.tile([128, NP], F32, tag="offz")
                        offy = p2k.tile([128, NP], F32, tag="offy")
                        offx = p2k.tile([128, NP], F32, tag="offx")
                        # de-interleave on ACT (keeps DVE free)
                        nc.scalar.activation(offz[:], sv[:, 0], AF.Copy)
                        nc.scalar.activation(offy[:], sv[:, 1], AF.Copy)
                        nc.scalar.activation(offx[:], sv[:, 2], AF.Copy)
                        # s_z = clamp(off_z + l, 0, 15) = min(Relu(off_z + l), 15)
                        nc.scalar.activation(offz[:], offz[:], AF.Relu,
                                             bias=lbt[:, lv:lv + 1])
                        nc.gpsimd.tensor_scalar(offz[:], offz[:], 15.0, None,
                                                ALU.min)
                        # f_y = clamp(off_y + gy + h0, 0, 95) - h0 - gy
                        nc.gpsimd.tensor_tensor(offy[:], offy[:], gyt[:], ALU.add)
                        nc.gpsimd.tensor_scalar(offy[:], offy[:], float(h0), 0.0,
                                                ALU.add, ALU.max)
                        nc.gpsimd.tensor_scalar(offy[:], offy[:], 95.0, float(h0),
                                                ALU.min, ALU.subtract)
                        nc.gpsimd.tensor_tensor(offy[:], offy[:], gyt[:],
                                                ALU.subtract)
                        # f_x = clamp(off_x + gx, 0, 95) - gx
                        nc.gpsimd.tensor_tensor(offx[:], offx[:], gxt[:], ALU.add)
                        nc.gpsimd.tensor_scalar(offx[:], offx[:], 0.0, 95.0,
                                                ALU.max, ALU.min)
                        nc.gpsimd.tensor_tensor(offx[:], offx[:], gxt[:],
                                                ALU.subtract)
                        # tents
                        u = p2k.tile([128, NP], BF16, tag="u")
                        # PWL knots: c_k = clamp(f_x - k, 0, 1), k in -2..1
                        cks, lamy = [], []
                        for ik, kk in enumerate((-2, -1, 0, 1)):
                            ck = p2k.tile([128, NP], BF16, tag=f"ck{ik}")
                            nc.scalar.activation(ck[:], offx[:], AF.Relu,
                                                 bias=bias_of[kk])
                            nc.gpsimd.tensor_scalar(ck[:], ck[:], 1.0, None,
                                                    ALU.min)
                            cks.append(ck)
                        for t in TAPS:
                            nc.scalar.activation(u[:], offy[:], AF.Abs,
                                                 bias=bias_of[t])
                            lt = p2k.tile([128, NP], BF16, tag=f"lamy{t}")
                            nc.scalar.activation(lt[:], u[:], AF.Relu,
                                                 bias=1.0, scale=neg1)
                            lamy.append(lt)
                        # interleaved-wave slot tiles (see wave emission
                        # below). offy/offx die after cks/lamy -> their
                        # f32 storage hosts 4 bf16 slots via bitcast.
                        lamz = p2k.tile([128, NP], BF16, tag="lamz")
                        accb = p2k.tile([128, NP], BF16, tag="accb")
                        offyB = offy[:].bitcast(BF16)
                        offxB = offx[:].bitcast(BF16)
                        sl0 = p2k.tile([128, NP], BF16, tag="tmpi")
                        sl1 = p2k.tile([128, NP], BF16, tag="prod")
                        sl2 = p2k.tile([128, NP], BF16, tag="tmpb")
                        sl3 = p2k.tile([128, NP], BF16, tag="tmpz")
                        sl4 = p2k.tile([128, NP], BF16, tag="nsl0")
                        sl5 = p2k.tile([128, NP], BF16, tag="nsl1")
                        sl6 = p2k.tile([128, NP], BF16, tag="nsl2")
                        sl7 = p2k.tile([128, NP], BF16, tag="nsl3")
                        sl8 = p2k.tile([128, NP], BF16, tag="nsl4")
                        sl9 = p2k.tile([128, NP], BF16, tag="nsl5")
                        t_pi = [[sl0[:], sl1[:]], [sl2[:], sl3[:]],
                                [offyB[:, 0:NP], offyB[:, NP:2 * NP]]]
                        t_tb = [[offxB[:, 0:NP], offxB[:, NP:2 * NP]],
                                [sl4[:], sl5[:]], [sl6[:], sl7[:]]]
                        t_pr = [sl8[:], sl9[:]]
                        # slots for wave-2's second z-stream (z-tap +2):
                        # x-PWL moved off GPSIMD (strided reads are slow
                        # on the software DSP) onto DVE as stream B
                        tmpi_g = p2k.tile([128, NP], BF16, tag="tmpi_g")
                        prod_g = p2k.tile([128, NP], BF16, tag="prod_g")
                        tmpb_g = p2k.tile([128, NP], BF16, tag="tmpb_g")
                        dzg = p2k.tile([128, 20, 99], BF16, tag="dzg")
                        # ---- wave 1: z-taps 0..2, 3 DVE streams ----
                        dzv = [slab[:, o:o + 1980].rearrange(
                                   "p (y x) -> p y x", y=20)
                               for o in (0, 1980, 3 * NP)]
                        for s in range(3):
                            nc.vector.tensor_tensor(
                                dzv[s], win[:, s, :, 1:100],
                                win[:, s, :, 0:99], ALU.subtract)
                        for iy, sy in enumerate(TAPS):
                            for ik in range(4):
                                def msl(s):
                                    return dzv[s][:, sy + 2:sy + 2 + HC,
                                                  ik:ik + W]

                                def asr(s):
                                    if ik == 0:
                                        return win[:, s, sy + 2:sy + 2 + HC,
                                                   0:W]
                                    return t_pi[s][(ik + 1) % 2]

                                nc.vector.tensor_tensor(
                                    t_pr[0], cks[ik][:], msl(0), ALU.mult)
                                nc.vector.tensor_tensor(
                                    t_pr[1], cks[ik][:], msl(1), ALU.mult)
                                nc.vector.tensor_tensor(
                                    t_pi[0][ik % 2], asr(0), t_pr[0],
                                    ALU.add)
                                nc.vector.tensor_tensor(
                                    t_pr[0], cks[ik][:], msl(2), ALU.mult)
                                nc.vector.tensor_tensor(
                                    t_pi[1][ik % 2], asr(1), t_pr[1],
                                    ALU.add)
                                nc.vector.tensor_tensor(
                                    t_pi[2][ik % 2], asr(2), t_pr[0],
                                    ALU.add)
                            fin = [t_pi[s][1] for s in range(3)]
                            if iy == 0:
                                for s in range(3):
                                    nc.vector.tensor_tensor(
                                        t_tb[s][0], lamy[0][:], fin[s],
                                        ALU.mult)
                            else:
                                nc.vector.tensor_tensor(
                                    t_pr[0], lamy[iy][:], fin[0], ALU.mult)
                                nc.vector.tensor_tensor(
                                    t_pr[1], lamy[iy][:], fin[1], ALU.mult)
                                nc.vector.tensor_tensor(
                                    t_tb[0][iy % 2], t_tb[0][(iy + 1) % 2],
                                    t_pr[0], ALU.add)
                                nc.vector.tensor_tensor(
                                    t_pr[0], lamy[iy][:], fin[2], ALU.mult)
                                nc.vector.tensor_tensor(
                                    t_tb[1][iy % 2], t_tb[1][(iy + 1) % 2],
                                    t_pr[1], ALU.add)
                                nc.vector.tensor_tensor(
                                    t_tb[2][iy % 2], t_tb[2][(iy + 1) % 2],
                                    t_pr[0], ALU.add)
                        # wave-1 z-products: lamz_z staggered on ACT
                        # (final tmpb of stream s lives in t_tb[s][0])
                        psl = [t_pi[0][0], t_pi[0][1], t_tb[0][1]]
                        for s in range(3):
                            nc.scalar.activation(
                                u[:], offz[:], AF.Abs,
                                bias=lbzt[:, lv * 5 + s:lv * 5 + s + 1])
                            nc.scalar.activation(lamz[:], u[:], AF.Relu,
                                                 bias=1.0, scale=neg1)
                            nc.vector.tensor_tensor(
                                psl[s], lamz[:], t_tb[s][0], ALU.mult)
                        s01 = t_tb[1][0]
                        s012 = t_tb[1][1]
                        nc.vector.tensor_tensor(s01, psl[0], psl[1],
                                                ALU.add)
                        nc.vector.tensor_tensor(s012, s01, psl[2], ALU.add)
                        # ---- wave 2: z-taps 3,4 as 2 interleaved z-streams
                        # stream A slots: t_pi[1] pair, tb pp t_pi[0];
                        # stream B slots: tmpi_g/prod_g pair, tb pp
                        # tmpb_g + dzg (flat view)
                        nc.vector.tensor_tensor(
                            dzv[0], win[:, 3, :, 1:100], win[:, 3, :, 0:99],
                            ALU.subtract)
                        nc.vector.tensor_tensor(
                            dzv[1], win[:, 4, :, 1:100], win[:, 4, :, 0:99],
                            ALU.subtract)
                        tgb = dzg[:].rearrange("p y x -> p (y x)")[:, 0:NP]
                        w2pi = [t_pi[1], [tmpi_g[:], prod_g[:]]]
                        w2tb = [t_pi[0], [tmpb_g[:], tgb]]
                        for iy, sy in enumerate(TAPS):
                            for ik in range(4):
                                for zz in range(2):
                                    nc.vector.tensor_tensor(
                                        t_pr[zz], cks[ik][:],
                                        dzv[zz][:, sy + 2:sy + 2 + HC,
                                                ik:ik + W], ALU.mult)
                                for zz in range(2):
                                    src = (win[:, 3 + zz,
                                               sy + 2:sy + 2 + HC, 0:W]
                                           if ik == 0
                                           else w2pi[zz][(ik + 1) % 2])
                                    nc.vector.tensor_tensor(
                                        w2pi[zz][ik % 2], src, t_pr[zz],
                                        ALU.add)
                            for zz in range(2):
                                if iy == 0:
                                    nc.vector.tensor_tensor(
                                        w2tb[zz][0], lamy[0][:],
                                        w2pi[zz][1], ALU.mult)
                                else:
                                    nc.vector.tensor_tensor(
                                        t_pr[zz], lamy[iy][:],
                                        w2pi[zz][1], ALU.mult)
                                    nc.vector.tensor_tensor(
                                        w2tb[zz][iy % 2],
                                        w2tb[zz][(iy + 1) % 2],
                                        t_pr[zz], ALU.add)
                        # final combine: + lamz3*tbA + lamz4*tbB
                        # (finals in w2tb[zz][0] since last iy=4 is even)
                        nc.scalar.activation(
                            u[:], offz[:], AF.Abs,
                            bias=lbzt[:, lv * 5 + 3:lv * 5 + 4])
                        nc.scalar.activation(lamz[:], u[:], AF.Relu,
                                             bias=1.0, scale=neg1)
                        nc.vector.tensor_tensor(
                            t_pi[1][0], lamz[:], w2tb[0][0], ALU.mult)
                        nc.vector.tensor_tensor(
                            t_pi[1][1], s012, t_pi[1][0], ALU.add)
                        nc.scalar.activation(
                            u[:], offz[:], AF.Abs,
                            bias=lbzt[:, lv * 5 + 4:lv * 5 + 5])
                        nc.scalar.activation(lamz[:], u[:], AF.Relu,
                                             bias=1.0, scale=neg1)
                        nc.vector.tensor_tensor(
                            t_pi[2][0], lamz[:], w2tb[1][0], ALU.mult)
                        nc.vector.tensor_tensor(
                            accb[:], t_pi[1][1], t_pi[2][0], ALU.add)
                        nc.sync.dma_start(
                            defds[lv][:, hc * HC * W:(hc + 1) * HC * W],
                            accb[:])

            # close P1 pools only now (after all P2 tags are allocated)
            # so P3 tiles reuse P1's SBUF, not P2's
            ctx1.close()
            # ---------- P3: banded partial main conv, overlapped with P2 ----
            p3w = ctx12.enter_context(tc.tile_pool(name="p3w", bufs=1))
            p3d = ctx12.enter_context(tc.tile_pool(name="p3d", bufs=2))
            p3ps = ctx12.enter_context(
                tc.tile_pool(name="p3ps", bufs=2, space="PSUM"))
            p3o = ctx12.enter_context(tc.tile_pool(name="p3o", bufs=2))
            if True:
                w3at = p3w.tile([128, 9, 64], BF16)
                nc.sync.dma_start(w3at[:], w3a.rearrange("g p m -> p g m"))
                w3bt = p3w.tile([128, 9, 64], BF16)
                nc.sync.dma_start(w3bt[:], w3b.rearrange("g p m -> p g m"))
                w3s2t = p3w.tile([64, 9, 64], BF16)
                nc.sync.dma_start(w3s2t[:], w3s2.rearrange("g p m -> p g m"))
                w3s0t = p3w.tile([64, 9, 64], BF16)
                nc.sync.dma_start(w3s0t[:], w3s0.rearrange("g p m -> p g m"))
                for hc3 in range(NHC):
                    y0 = HC * hc3
                    ys = max(0, y0 - 1)
                    ye = min(H, y0 + HC + 1)
                    for b in range(B):
                        dcA = p3d.tile([128, 18, 98], BF16, tag=f"dcA{b}")
                        nc.scalar.memzero(dcA[:].rearrange("p y x -> p (y x)"))
                        dcB = p3d.tile([64, 18, 98], BF16, tag=f"dcB{b}")
                        nc.scalar.memzero(dcB[:].rearrange("p y x -> p (y x)"))
                        for lv in range(2):
                            for dst in ([dcA[64 * lv:64 * lv + 64]]
                                        + ([dcB[0:64]] if lv == 1 else [])):
                                nc.sync.dma_start(
                                    dst[:, ys - y0 + 1:ye - y0 + 1, 1:97],
                                    defds[lv][b * 64:(b + 1) * 64,
                                              ys * W:ye * W]
                                    .rearrange("c (y x) -> c y x", y=ye - ys))
                        specs = [(w3s2t, dcA[0:64], 64),
                                 (w3at, dcA[:], 128),
                                 (w3bt, dcA[:], 128),
                                 (w3s0t, dcB[0:64], 64)]
                        for li, (wt, dct, kk) in enumerate(specs):
                            ob = p3o.tile([64, HC * W], F32,
                                          tag=f"ob3{li % 2}")
                            for qi, q0 in enumerate((0, 4, 8, 12)):
                                n = 4 * W
                                ps = p3ps.tile([64, 384], F32,
                                               tag=f"ps3{li % 2}")
                                g = 0
                                for dy in range(3):
                                    for dx in range(3):
                                        nc.tensor.matmul(
                                            ps[:, :n], wt[:, g, :],
                                            dct[:, dy + q0:dy + q0 + 4,
                                                dx:dx + W],
                                            start=(g == 0), stop=(g == 8))
                                        g += 1
                                nc.scalar.copy(
                                    ob[:, qi * n:(qi + 1) * n], ps[:, :n])
                            nc.sync.dma_start(
                                pout[li * 2 + b, :,
                                     y0 * W:y0 * W + HC * W], ob[:])
            ctx12.close()
    nc.finalize()
    return nc


def kernel(x, w_off, w_conv, b_conv):
    global _nc_cache
    import ml_dtypes
    x = np.asarray(x, dtype=np.float32)
    w_off = np.asarray(w_off, dtype=np.float32)
    w_conv = np.asarray(w_conv, dtype=np.float32)
    b_conv = np.asarray(b_conv, dtype=np.float32)

    if _nc_cache is None:
        _nc_cache = build_program()

    bf = ml_dtypes.bfloat16
    # P1 source: pad z/y/x by 1
    xp1 = np.zeros((B, C, L + 2, 98, 98), bf)
    xp1[:, :, 1:L + 1, 1:H + 1, 1:W + 1] = x.astype(bf)
    # P2 source: pad z/y/x by 2
    xp2 = np.zeros((B, C, L + 4, 100, 100), bf)
    xp2[:, :, 2:L + 2, 2:H + 2, 2:W + 2] = x.astype(bf)

    woff_r = w_off.reshape(64, 3, C, 3, 3, 3)      # [m', ch, c, dz, dy, dx]
    wt_off = np.ascontiguousarray(
        np.transpose(woff_r, (1, 4, 5, 3, 2, 0)))  # [ch, dy, dx, dz, c, m']
    wc_t = np.transpose(w_conv, (3, 4, 2, 1, 0))   # [dy, dx, dz, c, m]
    w3a = np.ascontiguousarray(
        wc_t[:, :, 1:3].reshape(9, 128, 64)).astype(bf)
    w3b = np.ascontiguousarray(
        wc_t[:, :, 0:2].reshape(9, 128, 64)).astype(bf)
    w3s2 = np.ascontiguousarray(wc_t[:, :, 2].reshape(9, 64, 64)).astype(bf)
    w3s0 = np.ascontiguousarray(wc_t[:, :, 0].reshape(9, 64, 64)).astype(bf)

    gyt = np.broadcast_to(
        np.repeat(np.arange(HC, dtype=np.float32), W)[None], (128, HC * W))
    gxt = np.broadcast_to(
        np.tile(np.arange(W, dtype=np.float32), HC)[None], (128, HC * W))
    gyt = np.ascontiguousarray(gyt).astype(bf)
    gxt = np.ascontiguousarray(gxt).astype(bf)

    in_maps = []
    for k in range(NCORES):
        units = [divmod(6 * k + j, 16) for j in range(6)]   # (ch, l2)
        x1 = np.empty((B, 6, 3, C, 98, 98), bf)
        for j, (ch, l2) in enumerate(units):
            x1[:, j] = np.transpose(xp1[:, :, l2:l2 + 3], (0, 2, 1, 3, 4))
        x2 = np.ascontiguousarray(
            xp2[:, :, 2 * k:2 * k + 6].reshape(128, 6, 100, 100))
        w1p = np.empty((6, 9, 128, 64), bf)
        w1s = np.empty((6, 9, 64, 64), bf)
        for j, (ch, l2) in enumerate(units):
            w1p[j] = wt_off[ch, :, :, 0:2].reshape(9, 128, 64)
            w1s[j] = wt_off[ch, :, :, 2].reshape(9, 64, 64)
        lbv = np.array([2 * k, 2 * k + 1], np.float32)
        lb = np.broadcast_to(lbv[None], (128, 2)).copy()
        # lbz[:, lv*5+iz] = -(2k + lv + sz), sz = TAPS[iz]
        lbzv = np.array([-(2 * k + lv + sz) for lv in range(2) for sz in TAPS],
                        np.float32)
        lbz = np.broadcast_to(lbzv[None], (128, 10)).copy()
        in_maps.append({
            "x1": x1, "x2": x2, "w1p": w1p, "w1s": w1s,
            "w3a": w3a, "w3b": w3b, "w3s2": w3s2, "w3s0": w3s0,
            "gy": gyt, "gx": gxt, "lb": lb, "lbz": lbz,
        })

    res = run_bass_kernel_spmd(_nc_cache, in_maps, list(range(NCORES)))

    out = np.zeros((B, 64, L, H, W), np.float32)
    for k in range(NCORES):
        po = res.results[k]["pout"]        # [8, 64, HW]
        for li in range(4):
            lg = 2 * k - 1 + li
            if 0 <= lg < L:
                for b in range(B):
                    out[b, :, lg] += po[li * 2 + b].reshape(64, H, W)
    out += b_conv[None, :, None, None, None]
    return out

